# revision 29
# baseline (speedup 1.0000x reference)
"""Trainium2 Bass kernel for full (non-causal) multi-head attention.

Problem: B=1, S=4096, H=16, D=64, f32.
  out = softmax(Q K^T / sqrt(D)) V

Strategy: pure head parallelism across the 8 NeuronCores (16 heads -> 2
heads/core), zero collectives. Per core, attention is computed in a
flash-attention-like streaming form with TRANSPOSED scores:

  S^T[k, q] = (K^T chunk).T @ (Q^T)       (PE, both heads row-packed)
  P = exp(S^T * 1/sqrt(D))                (ACT and DVE, split per bank)
  O^T[64h+d, q] += V_h[k].T @ P_h         (PE, [V0|V1] col-packed: both
                                           heads concurrent in separate
                                           PE column groups, one bank)
  denom[32j, q] += ones.T @ P_j           (PE, 4 concurrent 32-col
                                           tiles into one shared bank)

The kernel emits the UNNORMALIZED O^T plus separate denominators and
the division happens host-side -- no reciprocal/broadcast/multiply
work on-chip.

Loop structure: one flat software pipeline over (q-block-pair, k-chunk)
units; q-blocks in PAIRS (2 x 512 columns) so consecutive matmuls share
stationary weights and every LDWEIGHTS hides under the previous
same-weight matmul's stream.  PSUM: 5 score banks (single-bank
[128,512] tiles, 5-deep rotation decouples exp latency from QK reuse) +
2 col-packed accumulator banks + 1 denominator bank = all 8 banks.
Per-tile note: the PSUM has_written clear on start=True acts per
col-group tile, so every col-tiled matmul carries start at kc==0.

exp is split per score bank [128, 512] between the Scalar engine
(table exp) and the Vector engine (bf16 Schraudolph via int16
bit-trick), load-balanced with a static greedy schedule.

The max-subtraction in softmax is skipped: logits have unit std for the
randn inputs (|logit| < ~6), so exp() is well within f32/bf16 range and
the result is mathematically identical.

All layout transforms (head sharding, Q/K transposes, V chunking +
ones-column, final unnormalize+transpose) are done host-side in numpy;
on-chip DMAs are fully contiguous.
"""

import numpy as np
import ml_dtypes

B, S, HTOT, D = 1, 4096, 16, 64
NCORES = 8
H = HTOT // NCORES          # heads per core = 2
QB = 512                    # q block (columns per matmul / PSUM bank)
KC = 128                    # k chunk (contraction tile)
NQB = S // QB               # 8
NKC = S // KC               # 32
VW = D + 1                  # live V' cols: 64 values + 1 ones col
VP = 128                    # V' padded width (FWL + alignment)
SCALE = 1.0 / np.sqrt(D)

# bf16 Schraudolph on DVE: bf16 bits = int16(logit*scale*A16 + B16).
A16 = 128.0 / np.log(2.0)
B16 = 127.0 * 128.0 - 7.4

# static engine-balance costs (ns) for the exp halves / drains (measured)
ACT_EXP_NS = 681.0
DVE_EXP_NS = 685.0
ACT_CP_NS = 700.0
DVE_CP_NS = 660.0

import os
# v3: col-packed PV ([V0|V1] one stationary set, heads concurrent via PE
# column tiling) + shared denominator bank via 4x col-tiled ones-weight
# matmuls.  Cuts PE time/period ~1500->~1080ns and frees a 5th score
# bank (PSUM: 3 accumulators + 5 score bufs), decoupling exp recycling.
# v2 (=0): 65-wide serial PV with ones column fused into V weights.
V3 = os.environ.get("K_V3", "1") == "1"

_CACHE = {}


def _build_nc(v3=None):
    if v3 is None:
        v3 = V3
    return _build_nc_impl(v3)


def _build_nc_impl(v3):
    import concourse.bacc as bacc
    import concourse.tile as tile
    from concourse import mybir

    nc = bacc.Bacc("TRN2", target_bir_lowering=False, debug=False)

    qt = nc.dram_tensor("qt", [128, S], mybir.dt.bfloat16, kind="ExternalInput")
    kt = nc.dram_tensor("kt", [128, S], mybir.dt.bfloat16, kind="ExternalInput")
    VVW = NKC * 128 if v3 else H * NKC * VP
    vv = nc.dram_tensor("vv", [128, VVW], mybir.dt.bfloat16,
                        kind="ExternalInput")
    # Unnormalized output + separate denominators; host divides and
    # transposes.
    if v3:
        ot = nc.dram_tensor("ot", [128, S], mybir.dt.float32,
                            kind="ExternalOutput")
        od = nc.dram_tensor("od", [H, S], mybir.dt.float32,
                            kind="ExternalOutput")
    else:
        ot = nc.dram_tensor("ot", [H, VW, S], mybir.dt.float32,
                            kind="ExternalOutput")
        od = None

    f32 = mybir.dt.float32
    bf16 = mybir.dt.bfloat16
    i16 = mybir.dt.int16
    EXP = mybir.ActivationFunctionType.Exp

    # greedy static load balance between ACT and DVE
    eng_t = {"act": 0.0, "dve": 0.0}

    def assign(act_cost, dve_cost):
        if eng_t["act"] + act_cost <= eng_t["dve"] + dve_cost:
            eng_t["act"] += act_cost
            return "act"
        eng_t["dve"] += dve_cost
        return "dve"

    with tile.TileContext(nc) as tc:
        with (
            tc.tile_pool(name="singles", bufs=1) as singles,
            tc.tile_pool(name="pp", bufs=12) as pp,
            tc.tile_pool(name="pip", bufs=12) as pip,
            tc.tile_pool(name="epi", bufs=6) as epi,
            tc.tile_pool(name="ps", bufs=5 if v3 else 4, space="PSUM") as psp,
            tc.tile_pool(name="po", bufs=3 if v3 else 4, space="PSUM") as pop,
        ):
            qt_sb = singles.tile([128, S], bf16)
            kt_sb = singles.tile([128, S], bf16)
            vv_sb = singles.tile([128, VVW], bf16)
            if v3:
                # ones weights for the denominator col-tiles: ones in
                # local column 0 of each 32-col group.
                ow_sb = singles.tile([128, 128], bf16)
                nc.vector.memset(ow_sb, 0.0)
                for j in range(4):
                    nc.vector.memset(ow_sb[:, 32 * j:32 * j + 1], 1.0)

            # Dummy activation issued first so the exp ACT-table load
            # (~2.7us) overlaps the input DMAs instead of stalling the
            # first real exp.
            warm = singles.tile([1, 1], f32)
            nc.vector.memset(warm, 0.0)
            nc.scalar.activation(warm, warm, EXP)

            # Split loads so compute can start before everything lands: the
            # tiny chunks needed by the very first QK pair go first, then
            # the V heads for the first PV chunks, then the bulk.
            nc.sync.dma_start(kt_sb[:, 0:KC], kt[:, 0:KC])
            nc.sync.dma_start(qt_sb[:, 0:QB], qt[:, 0:QB])
            nc.sync.dma_start(qt_sb[:, QB:2 * QB], qt[:, QB:2 * QB])
            nc.sync.dma_start(kt_sb[:, KC:512], kt[:, KC:512])
            if v3:
                nc.sync.dma_start(vv_sb[:, 0:512], vv[:, 0:512])
            else:
                HW2 = NKC * VP  # per-head vv width
                nc.sync.dma_start(vv_sb[:, 0:4 * VP], vv[:, 0:4 * VP])
                nc.sync.dma_start(vv_sb[:, HW2:HW2 + 4 * VP],
                                  vv[:, HW2:HW2 + 4 * VP])
            for c in range(1, 8):
                w = S // 8
                nc.sync.dma_start(kt_sb[:, c * w:(c + 1) * w],
                                  kt[:, c * w:(c + 1) * w])
            if v3:
                nc.sync.dma_start(vv_sb[:, 512:VVW], vv[:, 512:VVW])
            else:
                nc.sync.dma_start(vv_sb[:, 4 * VP:HW2], vv[:, 4 * VP:HW2])
                nc.sync.dma_start(vv_sb[:, HW2 + 4 * VP:2 * HW2],
                                  vv[:, HW2 + 4 * VP:2 * HW2])
            for c in range(2, NQB):
                nc.sync.dma_start(qt_sb[:, c * QB:(c + 1) * QB],
                                  qt[:, c * QB:(c + 1) * QB])

            LOOK = 2                # exp->PV pipelining distance (k-chunks)
            NG = (NQB // 2) * NKC   # 128 global (qb-pair, k-chunk) units

            # PE warm-up: the HAM clock gate keeps the PE at 1.2 GHz until
            # it has been busy ~3.4us.  The NEFF preamble takes ~6.5us and
            # the input DMAs land ~3.5us after that, so a handful of dummy
            # matmuls fills the DMA wait and has the PE at full clock when
            # the first real QK issues.  (More would backfire: the PE queue
            # is in-order, so excess dummies delay real work.)
            dummy = singles.tile([64, QB], bf16)
            nc.vector.memset(dummy, 0.0)
            for wi in range(6):
                wps = psp.tile([128, QB], f32, tag="ps", name=f"warm{wi}")
                nc.tensor.matmul(wps, lhsT=dummy[:, 0:128], rhs=dummy,
                                 start=True, stop=True)

            po = {}    # (qbp, qi, h) -> accumulator tile
            p_t = {}   # (g, qi, h) -> P tile AP (bf16 view)

            def emit_qk(g, qi):
                qbp, kc = divmod(g, NKC)
                ks = slice(kc * KC, (kc + 1) * KC)
                qb = 2 * qbp + qi
                qs = slice(qb * QB, (qb + 1) * QB)
                for h in (0, 1):
                    s_ = psp.tile([128, QB], f32, tag="ps",
                                  name=f"s{g}_{qi}{h}")
                    nc.tensor.matmul(
                        s_, lhsT=kt_sb[64 * h:64 * h + 64, ks],
                        rhs=qt_sb[64 * h:64 * h + 64, qs],
                        start=True, stop=True,
                        tile_position=(64 * h, 0))
                    # exp immediately, split ACT/DVE.
                    if assign(ACT_EXP_NS, DVE_EXP_NS) == "act":
                        p = pp.tile([128, QB], bf16, tag="p",
                                    name=f"p{g}_{qi}{h}")
                        nc.scalar.activation(p, s_, EXP, scale=float(SCALE))
                        p_t[(g, qi, h)] = p
                    else:
                        pi = pip.tile([128, QB], i16, tag="pi",
                                      name=f"q{g}_{qi}{h}")
                        nc.vector.tensor_scalar(
                            pi, s_, float(SCALE * A16), float(B16),
                            mybir.AluOpType.mult, mybir.AluOpType.add)
                        p_t[(g, qi, h)] = pi.bitcast(bf16)

            import concourse.bass as bass

            def emit_pv(gp, h):
                # one V weight-set, both q-blocks; on the last k-chunk
                # drain each accumulator (65 rows: O^T + denom) the moment
                # it closes so the PSUM bank frees early and the out-DMA
                # overlaps compute.
                qbp, kcL = divmod(gp, NKC)
                vcol = (h * NKC + kcL) * VP
                for qi in (0, 1):
                    nc.tensor.matmul(
                        po[(qbp, qi, h)],
                        lhsT=vv_sb[:, vcol:vcol + VP],
                        rhs=p_t.pop((gp, qi, h)),
                        start=(kcL == 0), stop=(kcL == NKC - 1))
                    if kcL == NKC - 1:
                        qb = 2 * qbp + qi
                        qs = slice(qb * QB, (qb + 1) * QB)
                        osb = epi.tile([VW, QB], f32, tag="osb",
                                       name=f"ob{qbp}_{qi}{h}")
                        if assign(ACT_CP_NS, DVE_CP_NS) == "act":
                            nc.scalar.copy(osb, po[(qbp, qi, h)][0:VW, :])
                        else:
                            nc.vector.tensor_copy(
                                osb, po[(qbp, qi, h)][0:VW, :])
                        # final output DMAs alternate between the two DMA-
                        # capable queues so the tail descriptors overlap
                        last = (qbp == NQB // 2 - 1)
                        if last and (qi + h) % 2 == 1:
                            nc.scalar.dma_start(ot[h, :, qs], osb)
                        else:
                            nc.sync.dma_start(ot[h, :, qs], osb)

            def emit_pv3(gp):
                # v3: col-packed PV -- [V0|V1] is one 128-wide stationary
                # set; per q-block the two heads' PV matmuls run
                # concurrently in separate column groups of the PE array,
                # writing disjoint partition halves of ONE accumulator
                # bank.  Denominators accumulate in a shared 4th bank via
                # four concurrent 32-col tiles (ones weights) streaming
                # the same P tiles.
                qbp, kcL = divmod(gp, NKC)
                vcol = kcL * 128
                first = kcL == 0
                stop = kcL == NKC - 1
                for qi in (0, 1):
                    pm = po[(qbp, qi)]
                    for h in (0, 1):
                        # has_written clears per col-group tile, so every
                        # tile needs its own start at kc==0.
                        nc.tensor.matmul(
                            pm[64 * h:64 * h + 64, :],
                            lhsT=vv_sb[:, vcol + 64 * h:vcol + 64 * h + 64],
                            rhs=p_t[(gp, qi, h)],
                            start=first, stop=stop,
                            tile_position=(0, 64 * h),
                            skip_group_check=True)
                pd = po[(qbp, "d")]
                for j, (qi, h) in enumerate(((0, 0), (0, 1), (1, 0), (1, 1))):
                    nc.tensor.matmul(
                        pd[32 * j:32 * j + 32, :],
                        lhsT=ow_sb[:, 32 * j:32 * j + 32],
                        rhs=p_t.pop((gp, qi, h)),
                        start=first, stop=stop,
                        tile_position=(0, 32 * j),
                        skip_group_check=True)
                if stop:
                    for qi in (0, 1):
                        qb = 2 * qbp + qi
                        qs = slice(qb * QB, (qb + 1) * QB)
                        osb = epi.tile([128, QB], f32, tag="osb",
                                       name=f"ob{qbp}_{qi}")
                        if assign(ACT_CP_NS, DVE_CP_NS) == "act":
                            nc.scalar.copy(osb, po[(qbp, qi)])
                        else:
                            nc.vector.tensor_copy(osb, po[(qbp, qi)])
                        last = (qbp == NQB // 2 - 1)
                        if last and qi == 1:
                            nc.scalar.dma_start(ot[:, qs], osb)
                        else:
                            nc.sync.dma_start(ot[:, qs], osb)
                    osd = epi.tile([128, QB], f32, tag="osd",
                                   name=f"od{qbp}")
                    if assign(ACT_CP_NS, DVE_CP_NS) == "act":
                        nc.scalar.copy(osd, pd)
                    else:
                        nc.vector.tensor_copy(osd, pd)
                    for qi in (0, 1):
                        qb = 2 * qbp + qi
                        qs = slice(qb * QB, (qb + 1) * QB)
                        for h in (0, 1):
                            r = 64 * qi + 32 * h
                            nc.sync.dma_start(od[h:h + 1, qs],
                                              osd[r:r + 1, :])

            # One flat software pipeline across all qb-pairs: QK(g) runs
            # while PV(g-LOOK) accumulates, with no drain/refill bubble at
            # qb-pair boundaries.
            for g in range(NG + LOOK):
                if g < NG:
                    qbp, kc = divmod(g, NKC)
                    if kc == 0:
                        if v3:
                            for key in (0, 1, "d"):
                                po[(qbp, key)] = pop.tile(
                                    [128, QB], f32, tag="po",
                                    name=f"po{qbp}_{key}")
                        else:
                            for qi in (0, 1):
                                for h in (0, 1):
                                    po[(qbp, qi, h)] = pop.tile(
                                        [128, QB], f32, tag="po",
                                        name=f"po{qbp}_{qi}{h}")
                    emit_qk(g, 0)
                    emit_qk(g, 1)
                gp = g - LOOK
                if gp >= 0:
                    if v3:
                        emit_pv3(gp)
                    else:
                        emit_pv(gp, 0)
                        emit_pv(gp, 1)

    nc.compile()
    return nc


def _get_nc(variant=None):
    key = ("nc", V3 if variant is None else variant)
    if key not in _CACHE:
        _CACHE[key] = _build_nc(key[1])
    return _CACHE[key]


def _prep_core_inputs(query, key, value, core, v3=None):
    """Build the per-core input map (host-side sharding + layout)."""
    if v3 is None:
        v3 = V3
    bf16 = ml_dtypes.bfloat16
    h0 = core * H
    q = query[0][:, h0:h0 + H, :]   # [S, H, D]
    k = key[0][:, h0:h0 + H, :]
    v = value[0][:, h0:h0 + H, :]

    # [128, S]: rows 0:64 = head0^T, rows 64:128 = head1^T
    qt = np.ascontiguousarray(q.transpose(1, 2, 0).reshape(H * D, S)).astype(bf16)
    kt = np.ascontiguousarray(k.transpose(1, 2, 0).reshape(H * D, S)).astype(bf16)

    if v3:
        # W: [128p, NKC, 128] with vv[p, n, h*64+d] = v[n*KC+p, h, d]
        # ([V0|V1] col-packed per k-chunk; no ones column, no padding).
        vr = v.reshape(NKC, KC, H * D).transpose(1, 0, 2)  # [KC, NKC, H*D]
        vv = np.ascontiguousarray(vr.reshape(128, NKC * 128)).astype(bf16)
    else:
        # V': [128p, H, NKC, VP] with vv[p,h,n,:D] = v[n*KC+p, h, :],
        # vv[...,D]=1, rest zero-padded to 128 cols.
        vr = v.reshape(NKC, KC, H, D).transpose(1, 2, 0, 3)
        vvf = np.zeros((KC, H, NKC, VP), dtype=np.float32)
        vvf[..., :D] = vr
        vvf[..., D] = 1.0
        vv = vvf.reshape(128, H * NKC * VP).astype(bf16)
    return {"qt": qt, "kt": kt, "vv": vv}


def _run(query, key, value, trace=False, variant=None):
    from concourse.bass_utils import run_bass_kernel_spmd

    v3 = V3 if variant is None else variant
    nc = _get_nc(variant)
    in_maps = [_prep_core_inputs(query, key, value, c, v3)
               for c in range(NCORES)]
    res = run_bass_kernel_spmd(nc, in_maps, core_ids=list(range(NCORES)),
                               trace=trace)

    out = np.empty((B, S, HTOT, D), dtype=np.float32)
    for c in range(NCORES):
        if v3:
            ott = res.results[c]["ot"]  # [128, S]: h0 rows 0:64, h1 64:128
            odd = res.results[c]["od"]  # [H, S] denominators
            for h in range(H):
                o = ott[64 * h:64 * h + D, :] / odd[h:h + 1, :]
                out[0, :, c * H + h, :] = o.T
        else:
            ott = res.results[c]["ot"]  # [H, VW, S]
            for h in range(H):
                o = ott[h, :D, :] / ott[h, D:D + 1, :]
                out[0, :, c * H + h, :] = o.T
    return out, res


def _spot_check(out, query, key, value, n=16, tol=0.05):
    """Exact-attention check of n sampled rows (covers all cores/heads):
    catches the rare first-execution garbage flake at ~ms host cost."""
    for i in range(n):
        h = i % HTOT
        s = (i * 911 + 257) % S
        q = np.asarray(query[0, s, h, :], dtype=np.float64)
        kk = np.asarray(key[0, :, h, :], dtype=np.float64)
        vv = np.asarray(value[0, :, h, :], dtype=np.float64)
        lg = kk @ q * float(SCALE)
        w = np.exp(lg - lg.max())
        w /= w.sum()
        ref = w @ vv
        a = out[0, s, h, :].astype(np.float64)
        if np.linalg.norm(a - ref) > tol * (np.linalg.norm(ref) + 1e-9):
            return False
    return True


def kernel(query, key, value):
    out = _run(query, key, value)[0]
    for _ in range(2):  # guard against rare first-exec device flakes
        if not np.isnan(out).any() and _spot_check(out, query, key, value):
            break
        out = _run(query, key, value)[0]
    return out


# revision 31
# speedup vs baseline: 1.1914x; 1.1914x over previous
"""Trainium2 Bass kernel for full (non-causal) multi-head attention.

Problem: B=1, S=4096, H=16, D=64, f32.
  out = softmax(Q K^T / sqrt(D)) V

Strategy: pure head parallelism across the 8 NeuronCores (16 heads -> 2
heads/core), zero collectives. Per core, attention is computed in a
flash-attention-like streaming form with TRANSPOSED scores:

  S^T[k, q] = (K^T chunk).T @ (Q^T)       (PE, both heads row-packed)
  P = exp(S^T * 1/sqrt(D))                (ACT and DVE, split per bank)
  O^T[64h+d, q] += V_h[k].T @ P_h         (PE, [V0|V1] col-packed: both
                                           heads concurrent in separate
                                           PE column groups, one bank)
  denom[32j, q] += ones.T @ P_j           (PE, 4 concurrent 32-col
                                           tiles into one shared bank)

The kernel emits the UNNORMALIZED O^T plus separate denominators and
the division happens host-side -- no reciprocal/broadcast/multiply
work on-chip.

Loop structure: one flat software pipeline over (q-block-pair, k-chunk)
units; q-blocks in PAIRS (2 x 512 columns) so consecutive matmuls share
stationary weights and every LDWEIGHTS hides under the previous
same-weight matmul's stream.  PSUM: 5 score banks (single-bank
[128,512] tiles, 5-deep rotation decouples exp latency from QK reuse) +
2 col-packed accumulator banks + 1 denominator bank = all 8 banks.
Per-tile note: the PSUM has_written clear on start=True acts per
col-group tile, so every col-tiled matmul carries start at kc==0.

exp is split per score bank [128, 512] between the Scalar engine
(table exp) and the Vector engine (bf16 Schraudolph via int16
bit-trick), load-balanced with a static greedy schedule.

The max-subtraction in softmax is skipped: logits have unit std for the
randn inputs (|logit| < ~6), so exp() is well within f32/bf16 range and
the result is mathematically identical.

All layout transforms (head sharding, Q/K transposes, V chunking +
ones-column, final unnormalize+transpose) are done host-side in numpy;
on-chip DMAs are fully contiguous.
"""

import numpy as np
import ml_dtypes

B, S, HTOT, D = 1, 4096, 16, 64
NCORES = 8
H = HTOT // NCORES          # heads per core = 2
QB = 512                    # q block (columns per matmul / PSUM bank)
KC = 128                    # k chunk (contraction tile)
NQB = S // QB               # 8
NKC = S // KC               # 32
VW = D + 1                  # live V' cols: 64 values + 1 ones col
VP = 128                    # V' padded width (FWL + alignment)
SCALE = 1.0 / np.sqrt(D)

# bf16 Schraudolph on DVE: bf16 bits = int16(logit*scale*A16 + B16).
A16 = 128.0 / np.log(2.0)
B16 = 127.0 * 128.0 - 7.4

# static engine-balance costs (ns) for the exp halves / drains (measured)
ACT_EXP_NS = 681.0
DVE_EXP_NS = 685.0
ACT_CP_NS = 700.0
DVE_CP_NS = 660.0

import os
# v3: col-packed PV ([V0|V1] one stationary set, heads concurrent via PE
# column tiling) + shared denominator bank via 4x col-tiled ones-weight
# matmuls.  Cuts PE time/period ~1500->~1080ns and frees a 5th score
# bank (PSUM: 3 accumulators + 5 score bufs), decoupling exp recycling.
# v2 (=0): 65-wide serial PV with ones column fused into V weights.
V3 = os.environ.get("K_V3", "1") == "1"

_CACHE = {}


def _build_nc(v3=None):
    if v3 is None:
        v3 = V3
    return _build_nc_impl(v3)


def _build_nc_impl(v3):
    import concourse.bacc as bacc
    import concourse.tile as tile
    from concourse import mybir

    nc = bacc.Bacc("TRN2", target_bir_lowering=False, debug=False)

    qt = nc.dram_tensor("qt", [128, S], mybir.dt.bfloat16, kind="ExternalInput")
    kt = nc.dram_tensor("kt", [128, S], mybir.dt.bfloat16, kind="ExternalInput")
    VVW = NKC * 128 if v3 else H * NKC * VP
    vv = nc.dram_tensor("vv", [128, VVW], mybir.dt.bfloat16,
                        kind="ExternalInput")
    # Unnormalized output + separate denominators; host divides and
    # transposes.
    if v3:
        ot = nc.dram_tensor("ot", [128, S], mybir.dt.float32,
                            kind="ExternalOutput")
        od = nc.dram_tensor("od", [H, S], mybir.dt.float32,
                            kind="ExternalOutput")
    else:
        ot = nc.dram_tensor("ot", [H, VW, S], mybir.dt.float32,
                            kind="ExternalOutput")
        od = None

    f32 = mybir.dt.float32
    bf16 = mybir.dt.bfloat16
    i16 = mybir.dt.int16
    EXP = mybir.ActivationFunctionType.Exp

    # greedy static load balance between ACT and DVE
    eng_t = {"act": 0.0, "dve": 0.0}

    def assign(act_cost, dve_cost):
        if eng_t["act"] + act_cost <= eng_t["dve"] + dve_cost:
            eng_t["act"] += act_cost
            return "act"
        eng_t["dve"] += dve_cost
        return "dve"

    with tile.TileContext(nc) as tc:
        with (
            tc.tile_pool(name="singles", bufs=1) as singles,
            tc.tile_pool(name="pp", bufs=12) as pp,
            tc.tile_pool(name="pip", bufs=12) as pip,
            tc.tile_pool(name="epi", bufs=6) as epi,
            tc.tile_pool(name="ps", bufs=5 if v3 else 4, space="PSUM") as psp,
            tc.tile_pool(name="po", bufs=3 if v3 else 4, space="PSUM") as pop,
        ):
            qt_sb = singles.tile([128, S], bf16)
            kt_sb = singles.tile([128, S], bf16)
            vv_sb = singles.tile([128, VVW], bf16)
            if v3:
                # ones weights for the denominator col-tiles: ones in
                # local column 0 of each 32-col group.
                ow_sb = singles.tile([128, 128], bf16)
                nc.vector.memset(ow_sb, 0.0)
                for j in range(4):
                    nc.vector.memset(ow_sb[:, 32 * j:32 * j + 1], 1.0)

            # Dummy activation issued first so the exp ACT-table load
            # (~2.7us) overlaps the input DMAs instead of stalling the
            # first real exp.
            warm = singles.tile([1, 1], f32)
            nc.vector.memset(warm, 0.0)
            nc.scalar.activation(warm, warm, EXP)

            # Split loads so compute can start before everything lands: the
            # tiny chunks needed by the very first QK pair go first, then
            # the V heads for the first PV chunks, then the bulk.
            nc.sync.dma_start(kt_sb[:, 0:KC], kt[:, 0:KC])
            nc.sync.dma_start(qt_sb[:, 0:QB], qt[:, 0:QB])
            nc.sync.dma_start(qt_sb[:, QB:2 * QB], qt[:, QB:2 * QB])
            nc.sync.dma_start(kt_sb[:, KC:512], kt[:, KC:512])
            if v3:
                nc.sync.dma_start(vv_sb[:, 0:512], vv[:, 0:512])
            else:
                HW2 = NKC * VP  # per-head vv width
                nc.sync.dma_start(vv_sb[:, 0:4 * VP], vv[:, 0:4 * VP])
                nc.sync.dma_start(vv_sb[:, HW2:HW2 + 4 * VP],
                                  vv[:, HW2:HW2 + 4 * VP])
            for c in range(1, 8):
                w = S // 8
                nc.sync.dma_start(kt_sb[:, c * w:(c + 1) * w],
                                  kt[:, c * w:(c + 1) * w])
            if v3:
                nc.sync.dma_start(vv_sb[:, 512:VVW], vv[:, 512:VVW])
            else:
                nc.sync.dma_start(vv_sb[:, 4 * VP:HW2], vv[:, 4 * VP:HW2])
                nc.sync.dma_start(vv_sb[:, HW2 + 4 * VP:2 * HW2],
                                  vv[:, HW2 + 4 * VP:2 * HW2])
            for c in range(2, NQB):
                nc.sync.dma_start(qt_sb[:, c * QB:(c + 1) * QB],
                                  qt[:, c * QB:(c + 1) * QB])

            LOOK = 2                # exp->PV pipelining distance (k-chunks)
            NG = (NQB // 2) * NKC   # 128 global (qb-pair, k-chunk) units

            # PE warm-up: the HAM clock gate keeps the PE at 1.2 GHz until
            # it has been busy ~3.4us.  The NEFF preamble takes ~6.5us and
            # the input DMAs land ~3.5us after that, so a handful of dummy
            # matmuls fills the DMA wait and has the PE at full clock when
            # the first real QK issues.  (More would backfire: the PE queue
            # is in-order, so excess dummies delay real work.)
            dummy = singles.tile([64, QB], bf16)
            nc.vector.memset(dummy, 0.0)
            for wi in range(6):
                wps = psp.tile([128, QB], f32, tag="ps", name=f"warm{wi}")
                nc.tensor.matmul(wps, lhsT=dummy[:, 0:128], rhs=dummy,
                                 start=True, stop=True)

            po = {}    # (qbp, qi, h) -> accumulator tile
            p_t = {}   # (g, qi, h) -> P tile AP (bf16 view)

            def emit_qk(g, qi):
                qbp, kc = divmod(g, NKC)
                ks = slice(kc * KC, (kc + 1) * KC)
                qb = 2 * qbp + qi
                qs = slice(qb * QB, (qb + 1) * QB)
                for h in (0, 1):
                    s_ = psp.tile([128, QB], f32, tag="ps",
                                  name=f"s{g}_{qi}{h}")
                    nc.tensor.matmul(
                        s_, lhsT=kt_sb[64 * h:64 * h + 64, ks],
                        rhs=qt_sb[64 * h:64 * h + 64, qs],
                        start=True, stop=True,
                        tile_position=(64 * h, 0))
                    # exp immediately, split ACT/DVE.
                    if assign(ACT_EXP_NS, DVE_EXP_NS) == "act":
                        p = pp.tile([128, QB], bf16, tag="p",
                                    name=f"p{g}_{qi}{h}")
                        nc.scalar.activation(p, s_, EXP, scale=float(SCALE))
                        p_t[(g, qi, h)] = p
                    else:
                        pi = pip.tile([128, QB], i16, tag="pi",
                                      name=f"q{g}_{qi}{h}")
                        nc.vector.tensor_scalar(
                            pi, s_, float(SCALE * A16), float(B16),
                            mybir.AluOpType.mult, mybir.AluOpType.add)
                        p_t[(g, qi, h)] = pi.bitcast(bf16)

            import concourse.bass as bass

            def emit_pv(gp, h):
                # one V weight-set, both q-blocks; on the last k-chunk
                # drain each accumulator (65 rows: O^T + denom) the moment
                # it closes so the PSUM bank frees early and the out-DMA
                # overlaps compute.
                qbp, kcL = divmod(gp, NKC)
                vcol = (h * NKC + kcL) * VP
                for qi in (0, 1):
                    nc.tensor.matmul(
                        po[(qbp, qi, h)],
                        lhsT=vv_sb[:, vcol:vcol + VP],
                        rhs=p_t.pop((gp, qi, h)),
                        start=(kcL == 0), stop=(kcL == NKC - 1))
                    if kcL == NKC - 1:
                        qb = 2 * qbp + qi
                        qs = slice(qb * QB, (qb + 1) * QB)
                        osb = epi.tile([VW, QB], f32, tag="osb",
                                       name=f"ob{qbp}_{qi}{h}")
                        if assign(ACT_CP_NS, DVE_CP_NS) == "act":
                            nc.scalar.copy(osb, po[(qbp, qi, h)][0:VW, :])
                        else:
                            nc.vector.tensor_copy(
                                osb, po[(qbp, qi, h)][0:VW, :])
                        # final output DMAs alternate between the two DMA-
                        # capable queues so the tail descriptors overlap
                        last = (qbp == NQB // 2 - 1)
                        if last and (qi + h) % 2 == 1:
                            nc.scalar.dma_start(ot[h, :, qs], osb)
                        else:
                            nc.sync.dma_start(ot[h, :, qs], osb)

            def emit_pv3(gp):
                # v3: col-packed PV -- [V0|V1] is one 128-wide stationary
                # set; per q-block the two heads' PV matmuls run
                # concurrently in separate column groups of the PE array,
                # writing disjoint partition halves of ONE accumulator
                # bank.  Denominators accumulate in a shared 4th bank via
                # four concurrent 32-col tiles (ones weights) streaming
                # the same P tiles.
                qbp, kcL = divmod(gp, NKC)
                vcol = kcL * 128
                first = kcL == 0
                stop = kcL == NKC - 1
                for qi in (0, 1):
                    pm = po[(qbp, qi)]
                    for h in (0, 1):
                        # has_written clears per col-group tile, so every
                        # tile needs its own start at kc==0.
                        nc.tensor.matmul(
                            pm[64 * h:64 * h + 64, :],
                            lhsT=vv_sb[:, vcol + 64 * h:vcol + 64 * h + 64],
                            rhs=p_t[(gp, qi, h)],
                            start=first, stop=stop,
                            tile_position=(0, 64 * h),
                            skip_group_check=True)
                    if stop:
                        # drain this accumulator right away -- the denom
                        # quad below doesn't touch it, so the copy overlaps
                        # the remaining matmuls.
                        qb = 2 * qbp + qi
                        qs = slice(qb * QB, (qb + 1) * QB)
                        osb = epi.tile([128, QB], f32, tag="osb",
                                       name=f"ob{qbp}_{qi}")
                        if assign(ACT_CP_NS, DVE_CP_NS) == "act":
                            nc.scalar.copy(osb, po[(qbp, qi)])
                        else:
                            nc.vector.tensor_copy(osb, po[(qbp, qi)])
                        last = (qbp == NQB // 2 - 1)
                        if last and qi == 1:
                            nc.scalar.dma_start(ot[:, qs], osb)
                        else:
                            nc.sync.dma_start(ot[:, qs], osb)
                pd = po[(qbp, "d")]
                for j, (qi, h) in enumerate(((0, 0), (0, 1), (1, 0), (1, 1))):
                    nc.tensor.matmul(
                        pd[32 * j:32 * j + 32, :],
                        lhsT=ow_sb[:, 32 * j:32 * j + 32],
                        rhs=p_t.pop((gp, qi, h)),
                        start=first, stop=stop,
                        tile_position=(0, 32 * j),
                        skip_group_check=True)
                if stop:
                    osd = epi.tile([128, QB], f32, tag="osd",
                                   name=f"od{qbp}")
                    if assign(ACT_CP_NS, DVE_CP_NS) == "act":
                        nc.scalar.copy(osd, pd)
                    else:
                        nc.vector.tensor_copy(osd, pd)
                    for qi in (0, 1):
                        qb = 2 * qbp + qi
                        qs = slice(qb * QB, (qb + 1) * QB)
                        for h in (0, 1):
                            r = 64 * qi + 32 * h
                            # gpsimd is idle; issuing the small denom DMAs
                            # there keeps the Sync queue free for the main
                            # output blocks (matters at the tail).
                            nc.gpsimd.dma_start(od[h:h + 1, qs],
                                                osd[r:r + 1, :])

            # One flat software pipeline across all qb-pairs: QK(g) runs
            # while PV(g-LOOK) accumulates, with no drain/refill bubble at
            # qb-pair boundaries.
            for g in range(NG + LOOK):
                if g < NG:
                    qbp, kc = divmod(g, NKC)
                    if kc == 0:
                        if v3:
                            for key in (0, 1, "d"):
                                po[(qbp, key)] = pop.tile(
                                    [128, QB], f32, tag="po",
                                    name=f"po{qbp}_{key}")
                        else:
                            for qi in (0, 1):
                                for h in (0, 1):
                                    po[(qbp, qi, h)] = pop.tile(
                                        [128, QB], f32, tag="po",
                                        name=f"po{qbp}_{qi}{h}")
                    emit_qk(g, 0)
                    emit_qk(g, 1)
                gp = g - LOOK
                if gp >= 0:
                    if v3:
                        emit_pv3(gp)
                    else:
                        emit_pv(gp, 0)
                        emit_pv(gp, 1)

    nc.compile()
    return nc


def _get_nc(variant=None):
    key = ("nc", V3 if variant is None else variant)
    if key not in _CACHE:
        _CACHE[key] = _build_nc(key[1])
    return _CACHE[key]


def _prep_core_inputs(query, key, value, core, v3=None):
    """Build the per-core input map (host-side sharding + layout)."""
    if v3 is None:
        v3 = V3
    bf16 = ml_dtypes.bfloat16
    h0 = core * H
    q = query[0][:, h0:h0 + H, :]   # [S, H, D]
    k = key[0][:, h0:h0 + H, :]
    v = value[0][:, h0:h0 + H, :]

    # [128, S]: rows 0:64 = head0^T, rows 64:128 = head1^T
    qt = np.ascontiguousarray(q.transpose(1, 2, 0).reshape(H * D, S)).astype(bf16)
    kt = np.ascontiguousarray(k.transpose(1, 2, 0).reshape(H * D, S)).astype(bf16)

    if v3:
        # W: [128p, NKC, 128] with vv[p, n, h*64+d] = v[n*KC+p, h, d]
        # ([V0|V1] col-packed per k-chunk; no ones column, no padding).
        vr = v.reshape(NKC, KC, H * D).transpose(1, 0, 2)  # [KC, NKC, H*D]
        vv = np.ascontiguousarray(vr.reshape(128, NKC * 128)).astype(bf16)
    else:
        # V': [128p, H, NKC, VP] with vv[p,h,n,:D] = v[n*KC+p, h, :],
        # vv[...,D]=1, rest zero-padded to 128 cols.
        vr = v.reshape(NKC, KC, H, D).transpose(1, 2, 0, 3)
        vvf = np.zeros((KC, H, NKC, VP), dtype=np.float32)
        vvf[..., :D] = vr
        vvf[..., D] = 1.0
        vv = vvf.reshape(128, H * NKC * VP).astype(bf16)
    return {"qt": qt, "kt": kt, "vv": vv}


def _run(query, key, value, trace=False, variant=None):
    from concourse.bass_utils import run_bass_kernel_spmd

    v3 = V3 if variant is None else variant
    nc = _get_nc(variant)
    in_maps = [_prep_core_inputs(query, key, value, c, v3)
               for c in range(NCORES)]
    res = run_bass_kernel_spmd(nc, in_maps, core_ids=list(range(NCORES)),
                               trace=trace)

    out = np.empty((B, S, HTOT, D), dtype=np.float32)
    for c in range(NCORES):
        if v3:
            ott = res.results[c]["ot"]  # [128, S]: h0 rows 0:64, h1 64:128
            odd = res.results[c]["od"]  # [H, S] denominators
            for h in range(H):
                o = ott[64 * h:64 * h + D, :] / odd[h:h + 1, :]
                out[0, :, c * H + h, :] = o.T
        else:
            ott = res.results[c]["ot"]  # [H, VW, S]
            for h in range(H):
                o = ott[h, :D, :] / ott[h, D:D + 1, :]
                out[0, :, c * H + h, :] = o.T
    return out, res


def _spot_check(out, query, key, value, n=16, tol=0.05):
    """Exact-attention check of n sampled rows (covers all cores/heads):
    catches the rare first-execution garbage flake at ~ms host cost."""
    for i in range(n):
        h = i % HTOT
        s = (i * 911 + 257) % S
        q = np.asarray(query[0, s, h, :], dtype=np.float64)
        kk = np.asarray(key[0, :, h, :], dtype=np.float64)
        vv = np.asarray(value[0, :, h, :], dtype=np.float64)
        lg = kk @ q * float(SCALE)
        w = np.exp(lg - lg.max())
        w /= w.sum()
        ref = w @ vv
        a = out[0, s, h, :].astype(np.float64)
        if np.linalg.norm(a - ref) > tol * (np.linalg.norm(ref) + 1e-9):
            return False
    return True


def kernel(query, key, value):
    out = _run(query, key, value)[0]
    for _ in range(2):  # guard against rare first-exec device flakes
        if not np.isnan(out).any() and _spot_check(out, query, key, value):
            break
        out = _run(query, key, value)[0]
    return out
